# revision 22
# baseline (speedup 1.0000x reference)
"""Trainium2 Bass kernel for nn_PatchMMConvolution.

Computes a shared-weight 3x3 conv (stride 1, pad 1) over x[B=2, P=18, Cin=64,
H=128, W=128] with weight[Cout=128, Cin=64, 3, 3] + bias, i.e. conv2d on
36 images, returning [2, 18, 128, 128, 128] float32.

Strategy (8 NeuronCores, SPMD single program):
  - 36 images are split into 16 "streams" of 288 output rows each
    (2 full images + one quarter-image per stream). Each core runs two
    streams: stream A in SBUF partitions 0-63, stream B in partitions 64-127
    (Cin=64 channels live on partitions).
  - Host pre-pads each stream into a fp16 "slab" [64, 294, 130]: three
    vertically concatenated zero-padded segments (130+130+34 rows, W -> 130).
  - Conv is 9 shifted matmuls accumulating in PSUM. All wire data is fp16
    (inputs, weights, outputs); accumulation is fp32 in PSUM. fp16 halves
    HBM traffic and enables fast weight load (FWL) on the PE.
  - Loop: per 16-output-row chunk, 2 "superchunks" of 8 rows; within a
    superchunk the tap loop is OUTER. Each matmul is split into two M=64
    column-group matmuls (COL_SPLIT=2, explicit tile_position); streams A/B
    on PE row groups 0-1/2-3 run concurrently. PSUM: 4 tags x 2 bufs =
    8 banks (double buffered -- single-buffered PSUM stalls the PE on
    eviction bursts, measured +89us).
  - Eviction: Vector engine tensor_scalar_add(psum + bias) -> fp16 staging
    tile; one 0.5MB DMA per stream per chunk to DRAM.
  - A post-schedule BIR pass (DEDUP_LDW) drops the redundant second
    Ldweights the Tile scheduler emits per weight reuse pair.
  - Host upcasts the fp16 output to fp32.

Measured roofline notes (8x trn2, axon): an N=512 fp16 accumulating
matmul+LDW pair sustains ~264ns on its region chain regardless of region
packing, K/M size, LDW dedup, sem-inc stripping, AP shape/alignment,
accumulation flags, or active core count (micro.py) -- i.e. the PE
streams at ~2.0GHz effective here, making the per-core chain floor
648 x 264 = ~171us; the full kernel adds ~12us of pipeline edges.
"""

import numpy as np

import concourse.bass as bass
import concourse.mybir as mybir
import concourse.tile as tile
from concourse import bacc
from concourse._compat import get_trn_type
from concourse.bass_utils import run_bass_kernel_spmd

B, PP, CIN, H, W = 2, 18, 64, 128, 128
COUT = 128
NIMG = B * PP  # 36
NCORES = 8
NSTREAM = 16
WP = W + 2  # 130 padded width
RSLAB = 294  # 130 + 130 + 34 slab rows per stream
ROWS_PER_STREAM = 288
# (slab_row_base, out_row_base, out_rows) per segment
SEGS = [(0, 0, 128), (130, 128, 128), (260, 256, 32)]
CHUNK_OUT_ROWS = 16  # output rows per input chunk
CHUNK_ROWS = CHUNK_OUT_ROWS + 2  # 18 input rows per chunk
# Chunk sizing plan: "fixed" cuts every segment into CHUNK_OUT_ROWS pieces;
# "tapered" shrinks the first and last chunks of the rep so the startup
# input-DMA exposure and the end-of-rep eviction/output tail are shorter.
CHUNK_PLAN = "fixed"
TILE_OUT_ROWS = 4  # output rows per matmul tile (4*128 = 512 = one PSUM bank)
SC_TILES = 2  # matmul tiles per stream per superchunk (weight reuse factor)
SC_OUT_ROWS = SC_TILES * TILE_OUT_ROWS  # 8 output rows per superchunk

DT = mybir.dt.float16  # wire dtype for x and weights
ODT = mybir.dt.float16  # wire dtype for output
ACC = mybir.dt.float32

# Benchmark knob: repeat the whole kernel body KERNEL_REPS times inside a
# hardware loop (used to isolate device exec time from dispatch overhead).
KERNEL_REPS = 1
IN_BUFS = 5  # input chunk buffering depth
STG_BUFS = 2  # output staging buffering depth
# Ablation knobs (timing probes only; break correctness when True)
NO_EVICT = False  # skip PSUM->SBUF eviction (and bias)
NO_ODMA = False  # keep eviction but skip the output DMAs
NO_MM = False  # skip the matmuls
# Timing probe: emit only the first NTAPS of the 9 conv taps. Correctness
# holds only at 9; the slope of exec time vs NTAPS isolates per-tap PE cost
# from the fixed DMA/eviction/overhead intercept.
NTAPS = 9
# Split each matmul into COL_SPLIT column-group matmuls of M=128/COL_SPLIT.
# Smaller col regions shrink the per-region LDWEIGHTS exposure (P cols/1.2GHz)
# while the regions run concurrently on the PE. Measured: 2 is best (188us vs
# 197us at 1, 217us at 4 -- the 4-way split doubles instruction/semaphore
# overhead on the PE queue).
COL_SPLIT = 2
# Evict half the PSUM banks on the Scalar (ACT) engine instead of Vector:
# activation(Identity, bias=b, scale=1) does the same psum+bias -> fp16 copy.
ACT_EVICT = False
# Emit one explicit InstLdweights per (tap, stream, col-region) and order the
# two j-tile matmuls consecutively, probing whether walrus elides the
# per-matmul self-load when the same weights were just loaded.
EXPLICIT_LDW = False
# Matmul emission order within a tap (non-explicit path):
#   "j_outer":      j0:(A0,A1,B0,B1), j1:(A0,A1,B0,B1) -- distinct regions
#                   back-to-back so the FIFO PE queue never head-blocks.
#   "region_outer": A0:(j0,j1), A1:(j0,j1), ... -- consecutive matmuls share
#                   an identical weights AP, probing walrus self-load dedup.
MM_ORDER = "j_outer"
# Remove redundant Ldweights from the scheduled BIR. The Tile scheduler
# splits every matmul into an explicit Ldweights + Matmult pair (the ISA
# stream carries exactly one LDW per MMUL -- verified by neuron-disasm), so
# a matmul whose PE tile already holds the right weights pays a pointless
# ~53ns reload that cannot overlap its own tile's previous matmul. Weights
# stay resident per (row_grp, col_grp) tile until another Ldweights targets
# that same tile, so tracking the last-loaded weights AP per tile and
# dropping exact repeats (only those carrying no semaphore waits/updates)
# is sound.
DEDUP_LDW = True
# Strip the per-matmul completion increments Tile emits (every MMUL carries
# `$S++@complete`, serializing ~26ns of EVT_SEM traffic per matmul on the PE
# completion path). PE completions are in-order, so any wait at threshold v
# is safely replaced by the count of KEPT increments at the first kept
# matmul >= v; we keep only accumulation-group-final (stop=True) matmuls,
# which is exactly where Tile's real consumers (PSUM evictions) wait.
# Measured: no effect on HW exec time (the completion-inc path is not the
# bottleneck), so off by default.
STRIP_MM_INCS = False
# Fuse the two per-chunk output DMAs (stream A, stream B) into one DMA from
# a single combined staging tile [128, 2, rows, W]; halves dma_start count.
ODMA_FUSED = False
# Dummy N=512 matmuls emitted at the top of each rep, before any chunk
# dependency. After the loop barrier the PE would otherwise idle ~4us on
# the first chunk DMA -- one full HAM MID window -- so the real matmul
# stream starts throttled at 1.2GHz for ~3.4us. The dummies (garbage
# weights/rhs from w_sb into a scratch PSUM allocation, start=stop=True)
# keep the activity monitor busy through the DMA wait so real work starts
# at full clock.
WARMUP_MMS = 0

_PROGRAM = None


def _strip_mm_incs(nc):
    """Remove sem-incs from non-group-final Matmults; remap all waits and
    block-boundary add/sub/wait values on those semaphores. Returns the
    number of increments removed."""
    import concourse.mybir as _mybir

    # 1. Find, per (block, sem), the ordered list of Matmult incs.
    sem_blocks = {}  # sem_id -> block name with the MM incs
    mm_lists = {}  # sem_id -> list of (inst, kept)
    for f in nc.m.functions:
        for blk in f.blocks:
            for inst in blk.instructions:
                if inst.opcode != "Matmult" or not inst.sync_info:
                    continue
                for u in inst.sync_info.on_update:
                    if u.sync_type == "semaphore" and u.update_mode == "sem-inc":
                        bname = blk.name
                        if u.id in sem_blocks:
                            assert sem_blocks[u.id] == bname, (
                                f"sem {u.id} MM-incs span blocks"
                            )
                        else:
                            sem_blocks[u.id] = bname
                        mm_lists.setdefault(u.id, []).append(
                            (inst, bool(inst.stop_tensor_calc))
                        )
    removed = 0
    for sem_id, lst in mm_lists.items():
        total = len(lst)
        if total == 0 or all(k for _, k in lst):
            continue
        # kept-count prefix: kc[i] = #kept among first i incs (i in 0..total)
        kc = [0]
        for _, k in lst:
            kc.append(kc[-1] + (1 if k else 0))
        kept_total = kc[-1]
        assert kept_total > 0

        def remap(v):
            # first kept inc index m >= v (1-based); new threshold = kc[m]
            if v <= 0:
                return v
            if v > total:
                return v  # not an in-iteration threshold; handled by ==total
            m = v
            while m <= total and not lst[m - 1][1]:
                m += 1
            assert m <= total, f"wait {v} beyond last kept inc"
            return kc[m]

        # 2. Rewrite every wait/update on this sem everywhere.
        for f in nc.m.functions:
            for blk in f.blocks:
                for inst in blk.instructions:
                    si = inst.sync_info
                    if not si:
                        continue
                    for w in si.on_wait:
                        if w.sync_type == "semaphore" and w.id == sem_id:
                            assert w.wait_mode == "sem-ge-imm", str(w)
                            w.wait_value = (
                                kept_total
                                if w.wait_value == total
                                else remap(w.wait_value)
                            )
                    for u in si.on_update:
                        if u.sync_type != "semaphore" or u.id != sem_id:
                            continue
                        if u.update_mode in ("sem-add-imm", "sem-sub-imm"):
                            assert u.update_value == total, str(u)
                            u.update_value = kept_total
        # 3. Drop the non-kept incs.
        for inst, kept in lst:
            if kept:
                continue
            si = inst.sync_info
            keep_upd = [
                u
                for u in si.on_update
                if not (
                    u.sync_type == "semaphore"
                    and u.id == sem_id
                    and u.update_mode == "sem-inc"
                )
            ]
            inst.sync_info = _mybir.SyncInfo(
                on_wait=list(si.on_wait), on_update=keep_upd
            )
            removed += 1
    return removed


def _dedup_ldweights(nc):
    """Drop Ldweights whose (tile_position, weights-AP) matches the weights
    already resident in that PE tile. Tracking resets per basic block (the
    hardware loop body is one block, so cross-iteration carryover is never
    assumed). Returns the number of instructions removed."""
    removed = 0
    for f in nc.m.functions:
        for blk in f.blocks:
            insts = blk.instructions
            last = {}
            drop = []
            for idx in range(len(insts)):
                inst = insts[idx]
                if inst.opcode != "Ldweights":
                    continue
                tp = tuple(inst.tile_position or (0, 0))
                sig = (str(inst.ins[0]), inst.perf_mode, inst.is_transpose)
                si = inst.sync_info
                clean = si is None or (not si.on_wait and not si.on_update)
                if clean and last.get(tp) == sig:
                    drop.append(idx)
                else:
                    last[tp] = sig
            for idx in reversed(drop):
                del insts[idx]
            removed += len(drop)
    return removed


def _build_program():
    nc = bacc.Bacc(get_trn_type() or "TRN2", target_bir_lowering=False)
    xs = nc.dram_tensor("xs", [128, RSLAB, WP], DT, kind="ExternalInput")
    wd = nc.dram_tensor("wt", [128, 9, COUT], DT, kind="ExternalInput")
    bd = nc.dram_tensor("bias", [COUT, 1], ACC, kind="ExternalInput")
    od = nc.dram_tensor(
        "out", [COUT, 2, ROWS_PER_STREAM, W], ODT, kind="ExternalOutput"
    )

    if CHUNK_PLAN == "tapered":
        seg_sizes = [[8, 24] + [32] * 3, [32] * 4, [24, 8]]
    elif CHUNK_PLAN == "end8":
        seg_sizes = [
            [CHUNK_OUT_ROWS] * (128 // CHUNK_OUT_ROWS),
            [CHUNK_OUT_ROWS] * (128 // CHUNK_OUT_ROWS),
            [24, 8] if CHUNK_OUT_ROWS == 16 else [24, 8],
        ]
    else:
        seg_sizes = [
            [CHUNK_OUT_ROWS] * (nr // CHUNK_OUT_ROWS) for _, _, nr in SEGS
        ]
    chunks = []
    for (sb, ob, nr), sizes in zip(SEGS, seg_sizes):
        assert sum(sizes) == nr, (sizes, nr)
        off = 0
        for n in sizes:
            chunks.append((sb + off, ob + off, n))
            off += n

    psum_bufs = max(1, 8 // (2 * SC_TILES))
    with tile.TileContext(nc) as tc:
        with (
            tc.tile_pool(name="const", bufs=1) as cpool,
            tc.tile_pool(name="inp", bufs=IN_BUFS) as ipool,
            tc.tile_pool(name="stg", bufs=STG_BUFS) as spool,
            tc.tile_pool(name="ps", bufs=psum_bufs, space="PSUM") as pspool,
        ):
            w_sb = cpool.tile([128, 9, COUT], DT)
            nc.sync.dma_start(w_sb[:], wd[:])
            b_sb = cpool.tile([COUT, 1], ACC)
            nc.sync.dma_start(b_sb[:], bd[:])

            def emit_body():
                if WARMUP_MMS:
                    warm = pspool.tile(
                        [128, TILE_OUT_ROWS, W], ACC, tag="psA0", name="warm"
                    )
                    for _ in range(WARMUP_MMS):
                        nc.tensor.matmul(
                            warm[0:64],
                            w_sb[0:64, 0, 0:64],
                            w_sb[0:64, 0:TILE_OUT_ROWS, 0:W],
                            start=True,
                            stop=True,
                            tile_position=(0, 0),
                        )
                for srow, orow, nrows in chunks:
                    crows = nrows + 2
                    ch = ipool.tile([128, crows, WP], DT, tag="chunk")
                    nc.sync.dma_start(ch[:], xs[:, srow : srow + crows, :])
                    if ODMA_FUSED:
                        stAB = spool.tile([128, 2, nrows, W], ODT, tag="stAB")
                        stA = stAB[:, 0]
                        stB = stAB[:, 1]
                    else:
                        stA = spool.tile([128, nrows, W], ODT, tag="stA")
                        stB = spool.tile([128, nrows, W], ODT, tag="stB")
                    sc_out_rows = SC_TILES * TILE_OUT_ROWS
                    for k in range(nrows // sc_out_rows):
                        r0 = sc_out_rows * k
                        ps = [
                            [
                                pspool.tile(
                                    [128, TILE_OUT_ROWS, W],
                                    ACC,
                                    tag=f"ps{s}{j}",
                                    name=f"ps{s}{j}",
                                )
                                for j in range(SC_TILES)
                            ]
                            for s in ("A", "B")
                        ]
                        if not NO_MM:
                            mw = 128 // COL_SPLIT
                            for tap in range(NTAPS):
                                kh, kw = divmod(tap, 3)
                                first, last = tap == 0, tap == NTAPS - 1

                                def _rhs(s, j):
                                    rr = r0 + TILE_OUT_ROWS * j + kh
                                    return ch[
                                        64 * s : 64 * s + 64,
                                        rr : rr + TILE_OUT_ROWS,
                                        kw : kw + W,
                                    ]

                                def _wv(s, c):
                                    return w_sb[
                                        64 * s : 64 * s + 64,
                                        tap,
                                        c * mw : (c + 1) * mw,
                                    ]

                                if EXPLICIT_LDW:
                                    for s in range(2):
                                        for c in range(COL_SPLIT):
                                            nc.tensor.ldweights(
                                                _wv(s, c),
                                                tile_position=(64 * s, c * mw),
                                            )
                                            for j in range(SC_TILES):
                                                nc.tensor.matmul(
                                                    ps[s][j][c * mw : (c + 1) * mw],
                                                    _wv(s, c),
                                                    _rhs(s, j),
                                                    start=first,
                                                    stop=last,
                                                    tile_position=(64 * s, c * mw),
                                                )
                                elif MM_ORDER == "region_outer":
                                    for s in range(2):
                                        for c in range(COL_SPLIT):
                                            for j in range(SC_TILES):
                                                nc.tensor.matmul(
                                                    ps[s][j][c * mw : (c + 1) * mw],
                                                    _wv(s, c),
                                                    _rhs(s, j),
                                                    start=first,
                                                    stop=last,
                                                    tile_position=(64 * s, c * mw),
                                                )
                                else:
                                    # j-outer, stream/col inner: consecutive
                                    # matmuls hit distinct PE regions, so the
                                    # strict-FIFO PE queue never head-blocks
                                    # a free region behind a busy one.
                                    for j in range(SC_TILES):
                                        for s in range(2):
                                            for c in range(COL_SPLIT):
                                                nc.tensor.matmul(
                                                    ps[s][j][c * mw : (c + 1) * mw],
                                                    _wv(s, c),
                                                    _rhs(s, j),
                                                    start=first,
                                                    stop=last,
                                                    tile_position=(64 * s, c * mw),
                                                )
                        if not NO_EVICT:
                            # j-outer: eviction emission order matches the
                            # bank order the next superchunk's matmuls
                            # consume; with ACT_EVICT, consecutive banks
                            # alternate DVE/ACT so they drain pairwise.
                            for j in range(SC_TILES):
                                for s, stg in ((0, stA), (1, stB)):
                                    rr = r0 + TILE_OUT_ROWS * j
                                    dst = stg[:, rr : rr + TILE_OUT_ROWS, :]
                                    if ACT_EVICT and (2 * j + s) % 2 == 1:
                                        nc.scalar.activation(
                                            dst,
                                            ps[s][j][:],
                                            mybir.ActivationFunctionType.Identity,
                                            bias=b_sb[:],
                                            scale=1.0,
                                        )
                                    else:
                                        nc.vector.tensor_scalar_add(
                                            dst, ps[s][j][:], b_sb[:]
                                        )
                    if not NO_EVICT and not NO_ODMA:
                        if ODMA_FUSED:
                            nc.sync.dma_start(
                                od[:, :, orow : orow + nrows, :], stAB[:]
                            )
                        else:
                            nc.sync.dma_start(
                                od[:, 0, orow : orow + nrows, :], stA[:]
                            )
                            nc.sync.dma_start(
                                od[:, 1, orow : orow + nrows, :], stB[:]
                            )

            if KERNEL_REPS > 1:
                with tc.For_i(0, KERNEL_REPS, 1) as _i:
                    emit_body()
            else:
                emit_body()
    if DEDUP_LDW:
        _dedup_ldweights(nc)
    if STRIP_MM_INCS:
        _strip_mm_incs(nc)
    nc.finalize()
    return nc


def _get_program():
    global _PROGRAM
    if _PROGRAM is None:
        _PROGRAM = _build_program()
    return _PROGRAM


def _stream_parts(s):
    """Stream s covers full images 2s, 2s+1 and quarter (s%4) of image 32+(s//4)...
    returns (img0, img1, img_q, q) with quarter rows [32q, 32q+32)."""
    img_q = 32 + (s % 4)
    q = s // 4
    return 2 * s, 2 * s + 1, img_q, q


def _make_slab(X, s):
    """Build padded slab [CIN, RSLAB, WP] for stream s from X [NIMG,CIN,H,W]."""
    i0, i1, iq, q = _stream_parts(s)
    sl = np.zeros((CIN, RSLAB, WP), np.float16)
    sl[:, 1 : H + 1, 1 : W + 1] = X[i0]
    sl[:, 131 : 131 + H, 1 : W + 1] = X[i1]
    r0 = 32 * q
    lo, hi = max(r0 - 1, 0), min(r0 + 33, H)
    d0 = 260 + (lo - (r0 - 1))
    sl[:, d0 : d0 + (hi - lo), 1 : W + 1] = X[iq, :, lo:hi]
    return sl


def make_in_maps(x, weight, bias):
    x = np.asarray(x, dtype=np.float32)
    weight = np.asarray(weight, dtype=np.float32)
    bias = np.ascontiguousarray(np.asarray(bias), dtype=np.float32)
    X = x.reshape(NIMG, CIN, H, W).astype(np.float16)

    wt = np.ascontiguousarray(
        weight.transpose(1, 2, 3, 0).reshape(CIN, 9, COUT)
    ).astype(np.float16)
    wt2 = np.ascontiguousarray(np.concatenate([wt, wt], axis=0))  # [128, 9, COUT]
    bb = np.ascontiguousarray(bias.reshape(COUT, 1))

    in_maps = []
    for c in range(NCORES):
        xs = np.concatenate([_make_slab(X, 2 * c), _make_slab(X, 2 * c + 1)], axis=0)
        in_maps.append({"xs": np.ascontiguousarray(xs), "wt": wt2, "bias": bb})
    return in_maps


def kernel(x, weight, bias):
    in_maps = make_in_maps(x, weight, bias)
    nc = _get_program()
    res = run_bass_kernel_spmd(nc, in_maps, core_ids=list(range(NCORES)))

    Y = np.empty((NIMG, COUT, H, W), np.float32)
    for c in range(NCORES):
        o = res.results[c]["out"]  # [COUT, 2, 288, W] fp16
        for half in (0, 1):
            s = 2 * c + half
            i0, i1, iq, q = _stream_parts(s)
            oo = o[:, half].astype(np.float32)
            Y[i0] = oo[:, 0:H]
            Y[i1] = oo[:, H : 2 * H]
            Y[iq, :, 32 * q : 32 * q + 32, :] = oo[:, 2 * H : 2 * H + 32]
    return Y.reshape(B, PP, COUT, H, W)



# revision 27
# speedup vs baseline: 1.0060x; 1.0060x over previous
"""Trainium2 Bass kernel for nn_PatchMMConvolution.

Computes a shared-weight 3x3 conv (stride 1, pad 1) over x[B=2, P=18, Cin=64,
H=128, W=128] with weight[Cout=128, Cin=64, 3, 3] + bias, i.e. conv2d on
36 images, returning [2, 18, 128, 128, 128] float32.

Strategy (8 NeuronCores, SPMD single program):
  - 36 images are split into 16 "streams" of 288 output rows each
    (2 full images + one quarter-image per stream). Each core runs two
    streams: stream A in SBUF partitions 0-63, stream B in partitions 64-127
    (Cin=64 channels live on partitions).
  - Host pre-pads each stream into a fp16 "slab" [64, 294, 130]: three
    vertically concatenated zero-padded segments (130+130+34 rows, W -> 130).
  - Conv is 9 shifted matmuls accumulating in PSUM. All wire data is fp16
    (inputs, weights, outputs); accumulation is fp32 in PSUM. fp16 halves
    HBM traffic and enables fast weight load (FWL) on the PE.
  - Loop: per 16-output-row chunk, 2 "superchunks" of 8 rows; within a
    superchunk the tap loop is OUTER. Each matmul is split into two M=64
    column-group matmuls (COL_SPLIT=2, explicit tile_position); streams A/B
    on PE row groups 0-1/2-3 run concurrently. PSUM: 4 tags x 2 bufs =
    8 banks (double buffered -- single-buffered PSUM stalls the PE on
    eviction bursts, measured +89us).
  - Eviction: Vector engine tensor_scalar_add(psum + bias) -> fp16 staging
    tile; one 0.5MB DMA per stream per chunk to DRAM.
  - A post-schedule BIR pass (DEDUP_LDW) drops the redundant second
    Ldweights the Tile scheduler emits per weight reuse pair.
  - Host upcasts the fp16 output to fp32.

Measured roofline notes (8x trn2, axon): an N=512 fp16 accumulating
matmul+LDW pair sustains ~264ns on its region chain regardless of region
packing, K/M size, LDW dedup, sem-inc stripping, AP shape/alignment,
accumulation flags, or active core count (micro.py) -- i.e. the PE
streams at ~2.0GHz effective here, making the per-core chain floor
648 x 264 = ~171us; the full kernel adds ~12us of pipeline edges.
"""

import numpy as np

import concourse.bass as bass
import concourse.mybir as mybir
import concourse.tile as tile
from concourse import bacc
from concourse._compat import get_trn_type
from concourse.bass_utils import run_bass_kernel_spmd

B, PP, CIN, H, W = 2, 18, 64, 128, 128
COUT = 128
NIMG = B * PP  # 36
NCORES = 8
NSTREAM = 16
WP = W + 2  # 130 padded width
RSLAB = 294  # 130 + 130 + 34 slab rows per stream
ROWS_PER_STREAM = 288
# (slab_row_base, out_row_base, out_rows) per segment
SEGS = [(0, 0, 128), (130, 128, 128), (260, 256, 32)]
CHUNK_OUT_ROWS = 16  # output rows per input chunk
CHUNK_ROWS = CHUNK_OUT_ROWS + 2  # 18 input rows per chunk
# Chunk sizing plan: "fixed" cuts every segment into CHUNK_OUT_ROWS pieces;
# "tapered" shrinks the first and last chunks of the rep so the startup
# input-DMA exposure and the end-of-rep eviction/output tail are shorter.
CHUNK_PLAN = "fixed"
TILE_OUT_ROWS = 4  # output rows per matmul tile (4*128 = 512 = one PSUM bank)
SC_TILES = 2  # matmul tiles per stream per superchunk (weight reuse factor)
SC_OUT_ROWS = SC_TILES * TILE_OUT_ROWS  # 8 output rows per superchunk

DT = mybir.dt.float16  # wire dtype for x and weights
ODT = mybir.dt.float16  # wire dtype for output
ACC = mybir.dt.float32

# Benchmark knob: repeat the whole kernel body KERNEL_REPS times inside a
# hardware loop (used to isolate device exec time from dispatch overhead).
KERNEL_REPS = 1
IN_BUFS = 5  # input chunk buffering depth
STG_BUFS = 2  # output staging buffering depth
# Ablation knobs (timing probes only; break correctness when True)
NO_EVICT = False  # skip PSUM->SBUF eviction (and bias)
NO_ODMA = False  # keep eviction but skip the output DMAs
NO_MM = False  # skip the matmuls
# Timing probe: emit only the first NTAPS of the 9 conv taps. Correctness
# holds only at 9; the slope of exec time vs NTAPS isolates per-tap PE cost
# from the fixed DMA/eviction/overhead intercept.
NTAPS = 9
# Split each matmul into COL_SPLIT column-group matmuls of M=128/COL_SPLIT.
# Smaller col regions shrink the per-region LDWEIGHTS exposure (P cols/1.2GHz)
# while the regions run concurrently on the PE. Measured: 2 is best (188us vs
# 197us at 1, 217us at 4 -- the 4-way split doubles instruction/semaphore
# overhead on the PE queue).
COL_SPLIT = 2
# Evict half the PSUM banks on the Scalar (ACT) engine instead of Vector:
# activation(Identity, bias=b, scale=1) does the same psum+bias -> fp16 copy.
ACT_EVICT = False
# Emit one explicit InstLdweights per (tap, stream, col-region) and order the
# two j-tile matmuls consecutively, probing whether walrus elides the
# per-matmul self-load when the same weights were just loaded.
EXPLICIT_LDW = False
# Matmul emission order within a tap (non-explicit path):
#   "j_outer":      j0:(A0,A1,B0,B1), j1:(A0,A1,B0,B1) -- distinct regions
#                   back-to-back so the FIFO PE queue never head-blocks.
#   "region_outer": A0:(j0,j1), A1:(j0,j1), ... -- consecutive matmuls share
#                   an identical weights AP, probing walrus self-load dedup.
MM_ORDER = "j_outer"
# Remove redundant Ldweights from the scheduled BIR. The Tile scheduler
# splits every matmul into an explicit Ldweights + Matmult pair (the ISA
# stream carries exactly one LDW per MMUL -- verified by neuron-disasm), so
# a matmul whose PE tile already holds the right weights pays a pointless
# ~53ns reload that cannot overlap its own tile's previous matmul. Weights
# stay resident per (row_grp, col_grp) tile until another Ldweights targets
# that same tile, so tracking the last-loaded weights AP per tile and
# dropping exact repeats (only those carrying no semaphore waits/updates)
# is sound.
DEDUP_LDW = True
# Strip the per-matmul completion increments Tile emits (every MMUL carries
# `$S++@complete`, serializing ~26ns of EVT_SEM traffic per matmul on the PE
# completion path). PE completions are in-order, so any wait at threshold v
# is safely replaced by the count of KEPT increments at the first kept
# matmul >= v; we keep only accumulation-group-final (stop=True) matmuls,
# which is exactly where Tile's real consumers (PSUM evictions) wait.
# Measured: no effect on HW exec time (the completion-inc path is not the
# bottleneck), so off by default.
STRIP_MM_INCS = False
# Fuse the two per-chunk output DMAs (stream A, stream B) into one DMA from
# a single combined staging tile [128, 2, rows, W]; halves dma_start count.
ODMA_FUSED = False
# Dummy N=512 matmuls emitted at the top of each rep, before any chunk
# dependency. After the loop barrier the PE would otherwise idle ~4us on
# the first chunk DMA -- one full HAM MID window -- so the real matmul
# stream starts throttled at 1.2GHz for ~3.4us. The dummies (garbage
# weights/rhs from w_sb into a scratch PSUM allocation, start=stop=True)
# keep the activity monitor busy through the DMA wait so real work starts
# at full clock.
WARMUP_MMS = 0
# Keep the whole input slab resident in SBUF (76KB/partition of the 208KB
# budget) instead of streaming 18-row chunks through a 5-deep ring. The
# slab is filled by a handful of band DMAs issued up-front with no
# buffer-reuse WAR dependencies, halo rows are not re-read, and Tile's
# subtile tracking gates each matmul only on the band DMA that wrote its
# rows.
SLAB_RESIDENT = False
SLAB_BANDS = [18] + [46] * 6  # 294 rows; small first band -> early start
# Raise Tile-scheduler priority of input chunk DMAs so they are scheduled
# as if issued at program start (earlier prefetch in the SP queue).
HIPRIO_IN_DMA = False

_PROGRAM = None


def _strip_mm_incs(nc):
    """Remove sem-incs from non-group-final Matmults; remap all waits and
    block-boundary add/sub/wait values on those semaphores. Returns the
    number of increments removed."""
    import concourse.mybir as _mybir

    # 1. Find, per (block, sem), the ordered list of Matmult incs.
    sem_blocks = {}  # sem_id -> block name with the MM incs
    mm_lists = {}  # sem_id -> list of (inst, kept)
    for f in nc.m.functions:
        for blk in f.blocks:
            for inst in blk.instructions:
                if inst.opcode != "Matmult" or not inst.sync_info:
                    continue
                for u in inst.sync_info.on_update:
                    if u.sync_type == "semaphore" and u.update_mode == "sem-inc":
                        bname = blk.name
                        if u.id in sem_blocks:
                            assert sem_blocks[u.id] == bname, (
                                f"sem {u.id} MM-incs span blocks"
                            )
                        else:
                            sem_blocks[u.id] = bname
                        mm_lists.setdefault(u.id, []).append(
                            (inst, bool(inst.stop_tensor_calc))
                        )
    removed = 0
    for sem_id, lst in mm_lists.items():
        total = len(lst)
        if total == 0 or all(k for _, k in lst):
            continue
        # kept-count prefix: kc[i] = #kept among first i incs (i in 0..total)
        kc = [0]
        for _, k in lst:
            kc.append(kc[-1] + (1 if k else 0))
        kept_total = kc[-1]
        assert kept_total > 0

        def remap(v):
            # first kept inc index m >= v (1-based); new threshold = kc[m]
            if v <= 0:
                return v
            if v > total:
                return v  # not an in-iteration threshold; handled by ==total
            m = v
            while m <= total and not lst[m - 1][1]:
                m += 1
            assert m <= total, f"wait {v} beyond last kept inc"
            return kc[m]

        # 2. Rewrite every wait/update on this sem everywhere.
        for f in nc.m.functions:
            for blk in f.blocks:
                for inst in blk.instructions:
                    si = inst.sync_info
                    if not si:
                        continue
                    for w in si.on_wait:
                        if w.sync_type == "semaphore" and w.id == sem_id:
                            assert w.wait_mode == "sem-ge-imm", str(w)
                            w.wait_value = (
                                kept_total
                                if w.wait_value == total
                                else remap(w.wait_value)
                            )
                    for u in si.on_update:
                        if u.sync_type != "semaphore" or u.id != sem_id:
                            continue
                        if u.update_mode in ("sem-add-imm", "sem-sub-imm"):
                            assert u.update_value == total, str(u)
                            u.update_value = kept_total
        # 3. Drop the non-kept incs.
        for inst, kept in lst:
            if kept:
                continue
            si = inst.sync_info
            keep_upd = [
                u
                for u in si.on_update
                if not (
                    u.sync_type == "semaphore"
                    and u.id == sem_id
                    and u.update_mode == "sem-inc"
                )
            ]
            inst.sync_info = _mybir.SyncInfo(
                on_wait=list(si.on_wait), on_update=keep_upd
            )
            removed += 1
    return removed


def _dedup_ldweights(nc):
    """Drop Ldweights whose (tile_position, weights-AP) matches the weights
    already resident in that PE tile. Tracking resets per basic block (the
    hardware loop body is one block, so cross-iteration carryover is never
    assumed). Returns the number of instructions removed."""
    removed = 0
    for f in nc.m.functions:
        for blk in f.blocks:
            insts = blk.instructions
            last = {}
            drop = []
            for idx in range(len(insts)):
                inst = insts[idx]
                if inst.opcode != "Ldweights":
                    continue
                tp = tuple(inst.tile_position or (0, 0))
                sig = (str(inst.ins[0]), inst.perf_mode, inst.is_transpose)
                si = inst.sync_info
                clean = si is None or (not si.on_wait and not si.on_update)
                if clean and last.get(tp) == sig:
                    drop.append(idx)
                else:
                    last[tp] = sig
            for idx in reversed(drop):
                del insts[idx]
            removed += len(drop)
    return removed


def _build_program():
    nc = bacc.Bacc(get_trn_type() or "TRN2", target_bir_lowering=False)
    xs = nc.dram_tensor("xs", [128, RSLAB, WP], DT, kind="ExternalInput")
    wd = nc.dram_tensor("wt", [128, 9, COUT], DT, kind="ExternalInput")
    bd = nc.dram_tensor("bias", [COUT, 1], ACC, kind="ExternalInput")
    od = nc.dram_tensor(
        "out", [COUT, 2, ROWS_PER_STREAM, W], ODT, kind="ExternalOutput"
    )

    if CHUNK_PLAN == "tapered":
        seg_sizes = [[8, 24] + [32] * 3, [32] * 4, [24, 8]]
    elif CHUNK_PLAN == "end8":
        seg_sizes = [
            [CHUNK_OUT_ROWS] * (128 // CHUNK_OUT_ROWS),
            [CHUNK_OUT_ROWS] * (128 // CHUNK_OUT_ROWS),
            [24, 8] if CHUNK_OUT_ROWS == 16 else [24, 8],
        ]
    else:
        seg_sizes = [
            [CHUNK_OUT_ROWS] * (nr // CHUNK_OUT_ROWS) for _, _, nr in SEGS
        ]
    chunks = []
    for (sb, ob, nr), sizes in zip(SEGS, seg_sizes):
        assert sum(sizes) == nr, (sizes, nr)
        off = 0
        for n in sizes:
            chunks.append((sb + off, ob + off, n))
            off += n

    psum_bufs = max(1, 8 // (2 * SC_TILES))
    with tile.TileContext(nc) as tc:
        with (
            tc.tile_pool(name="const", bufs=1) as cpool,
            tc.tile_pool(name="inp", bufs=IN_BUFS) as ipool,
            tc.tile_pool(name="stg", bufs=STG_BUFS) as spool,
            tc.tile_pool(name="ps", bufs=psum_bufs, space="PSUM") as pspool,
        ):
            w_sb = cpool.tile([128, 9, COUT], DT)
            nc.sync.dma_start(w_sb[:], wd[:])
            b_sb = cpool.tile([COUT, 1], ACC)
            nc.sync.dma_start(b_sb[:], bd[:])

            def emit_body():
                slab = None
                if SLAB_RESIDENT:
                    slab = cpool.tile([128, RSLAB, WP], DT, tag="slab")
                    r = 0
                    for nb in SLAB_BANDS:
                        nc.sync.dma_start(
                            slab[:, r : r + nb, :], xs[:, r : r + nb, :]
                        )
                        r += nb
                    assert r == RSLAB
                if WARMUP_MMS:
                    warm = pspool.tile(
                        [128, TILE_OUT_ROWS, W], ACC, tag="psA0", name="warm"
                    )
                    for _ in range(WARMUP_MMS):
                        nc.tensor.matmul(
                            warm[0:64],
                            w_sb[0:64, 0, 0:64],
                            w_sb[0:64, 0:TILE_OUT_ROWS, 0:W],
                            start=True,
                            stop=True,
                            tile_position=(0, 0),
                        )
                for srow, orow, nrows in chunks:
                    crows = nrows + 2
                    if SLAB_RESIDENT:
                        ch = slab[:, srow : srow + crows, :]
                    else:
                        ch = ipool.tile([128, crows, WP], DT, tag="chunk")
                        if HIPRIO_IN_DMA:
                            with tc.high_priority():
                                nc.sync.dma_start(
                                    ch[:], xs[:, srow : srow + crows, :]
                                )
                        else:
                            nc.sync.dma_start(
                                ch[:], xs[:, srow : srow + crows, :]
                            )
                    if ODMA_FUSED:
                        stAB = spool.tile([128, 2, nrows, W], ODT, tag="stAB")
                        stA = stAB[:, 0]
                        stB = stAB[:, 1]
                    else:
                        stA = spool.tile([128, nrows, W], ODT, tag="stA")
                        stB = spool.tile([128, nrows, W], ODT, tag="stB")
                    sc_out_rows = SC_TILES * TILE_OUT_ROWS
                    for k in range(nrows // sc_out_rows):
                        r0 = sc_out_rows * k
                        ps = [
                            [
                                pspool.tile(
                                    [128, TILE_OUT_ROWS, W],
                                    ACC,
                                    tag=f"ps{s}{j}",
                                    name=f"ps{s}{j}",
                                )
                                for j in range(SC_TILES)
                            ]
                            for s in ("A", "B")
                        ]
                        if not NO_MM:
                            mw = 128 // COL_SPLIT
                            for tap in range(NTAPS):
                                kh, kw = divmod(tap, 3)
                                first, last = tap == 0, tap == NTAPS - 1

                                def _rhs(s, j):
                                    rr = r0 + TILE_OUT_ROWS * j + kh
                                    return ch[
                                        64 * s : 64 * s + 64,
                                        rr : rr + TILE_OUT_ROWS,
                                        kw : kw + W,
                                    ]

                                def _wv(s, c):
                                    return w_sb[
                                        64 * s : 64 * s + 64,
                                        tap,
                                        c * mw : (c + 1) * mw,
                                    ]

                                if EXPLICIT_LDW:
                                    for s in range(2):
                                        for c in range(COL_SPLIT):
                                            nc.tensor.ldweights(
                                                _wv(s, c),
                                                tile_position=(64 * s, c * mw),
                                            )
                                            for j in range(SC_TILES):
                                                nc.tensor.matmul(
                                                    ps[s][j][c * mw : (c + 1) * mw],
                                                    _wv(s, c),
                                                    _rhs(s, j),
                                                    start=first,
                                                    stop=last,
                                                    tile_position=(64 * s, c * mw),
                                                )
                                elif MM_ORDER == "region_outer":
                                    for s in range(2):
                                        for c in range(COL_SPLIT):
                                            for j in range(SC_TILES):
                                                nc.tensor.matmul(
                                                    ps[s][j][c * mw : (c + 1) * mw],
                                                    _wv(s, c),
                                                    _rhs(s, j),
                                                    start=first,
                                                    stop=last,
                                                    tile_position=(64 * s, c * mw),
                                                )
                                else:
                                    # j-outer, stream/col inner: consecutive
                                    # matmuls hit distinct PE regions, so the
                                    # strict-FIFO PE queue never head-blocks
                                    # a free region behind a busy one.
                                    for j in range(SC_TILES):
                                        for s in range(2):
                                            for c in range(COL_SPLIT):
                                                nc.tensor.matmul(
                                                    ps[s][j][c * mw : (c + 1) * mw],
                                                    _wv(s, c),
                                                    _rhs(s, j),
                                                    start=first,
                                                    stop=last,
                                                    tile_position=(64 * s, c * mw),
                                                )
                        if not NO_EVICT:
                            # j-outer: eviction emission order matches the
                            # bank order the next superchunk's matmuls
                            # consume; with ACT_EVICT, consecutive banks
                            # alternate DVE/ACT so they drain pairwise.
                            for j in range(SC_TILES):
                                for s, stg in ((0, stA), (1, stB)):
                                    rr = r0 + TILE_OUT_ROWS * j
                                    dst = stg[:, rr : rr + TILE_OUT_ROWS, :]
                                    if ACT_EVICT and (2 * j + s) % 2 == 1:
                                        nc.scalar.activation(
                                            dst,
                                            ps[s][j][:],
                                            mybir.ActivationFunctionType.Identity,
                                            bias=b_sb[:],
                                            scale=1.0,
                                        )
                                    else:
                                        nc.vector.tensor_scalar_add(
                                            dst, ps[s][j][:], b_sb[:]
                                        )
                    if not NO_EVICT and not NO_ODMA:
                        if ODMA_FUSED:
                            nc.sync.dma_start(
                                od[:, :, orow : orow + nrows, :], stAB[:]
                            )
                        else:
                            nc.sync.dma_start(
                                od[:, 0, orow : orow + nrows, :], stA[:]
                            )
                            nc.sync.dma_start(
                                od[:, 1, orow : orow + nrows, :], stB[:]
                            )

            if KERNEL_REPS > 1:
                with tc.For_i(0, KERNEL_REPS, 1) as _i:
                    emit_body()
            else:
                emit_body()
    if DEDUP_LDW:
        _dedup_ldweights(nc)
    if STRIP_MM_INCS:
        _strip_mm_incs(nc)
    nc.finalize()
    return nc


def _get_program():
    global _PROGRAM
    if _PROGRAM is None:
        _PROGRAM = _build_program()
    return _PROGRAM


def _stream_parts(s):
    """Stream s covers full images 2s, 2s+1 and quarter (s%4) of image 32+(s//4)...
    returns (img0, img1, img_q, q) with quarter rows [32q, 32q+32)."""
    img_q = 32 + (s % 4)
    q = s // 4
    return 2 * s, 2 * s + 1, img_q, q


def _make_slab(X, s):
    """Build padded slab [CIN, RSLAB, WP] for stream s from X [NIMG,CIN,H,W]."""
    i0, i1, iq, q = _stream_parts(s)
    sl = np.zeros((CIN, RSLAB, WP), np.float16)
    sl[:, 1 : H + 1, 1 : W + 1] = X[i0]
    sl[:, 131 : 131 + H, 1 : W + 1] = X[i1]
    r0 = 32 * q
    lo, hi = max(r0 - 1, 0), min(r0 + 33, H)
    d0 = 260 + (lo - (r0 - 1))
    sl[:, d0 : d0 + (hi - lo), 1 : W + 1] = X[iq, :, lo:hi]
    return sl


def make_in_maps(x, weight, bias):
    x = np.asarray(x, dtype=np.float32)
    weight = np.asarray(weight, dtype=np.float32)
    bias = np.ascontiguousarray(np.asarray(bias), dtype=np.float32)
    X = x.reshape(NIMG, CIN, H, W).astype(np.float16)

    wt = np.ascontiguousarray(
        weight.transpose(1, 2, 3, 0).reshape(CIN, 9, COUT)
    ).astype(np.float16)
    wt2 = np.ascontiguousarray(np.concatenate([wt, wt], axis=0))  # [128, 9, COUT]
    bb = np.ascontiguousarray(bias.reshape(COUT, 1))

    in_maps = []
    for c in range(NCORES):
        xs = np.concatenate([_make_slab(X, 2 * c), _make_slab(X, 2 * c + 1)], axis=0)
        in_maps.append({"xs": np.ascontiguousarray(xs), "wt": wt2, "bias": bb})
    return in_maps


def kernel(x, weight, bias):
    in_maps = make_in_maps(x, weight, bias)
    nc = _get_program()
    res = run_bass_kernel_spmd(nc, in_maps, core_ids=list(range(NCORES)))

    Y = np.empty((NIMG, COUT, H, W), np.float32)
    for c in range(NCORES):
        o = res.results[c]["out"]  # [COUT, 2, 288, W] fp16
        for half in (0, 1):
            s = 2 * c + half
            i0, i1, iq, q = _stream_parts(s)
            oo = o[:, half].astype(np.float32)
            Y[i0] = oo[:, 0:H]
            Y[i1] = oo[:, H : 2 * H]
            Y[iq, :, 32 * q : 32 * q + 32, :] = oo[:, 2 * H : 2 * H + 32]
    return Y.reshape(B, PP, COUT, H, W)

